# revision 13
# baseline (speedup 1.0000x reference)
# Trainium2 Bass kernel for nn_Block_9483287789889 (dense transformer block).
#
# Sharding (8 cores): 2 cores per batch (B=4). Host permutes each batch's
# 2048 tokens into [owned 8x128-tiles (interleaved) | other 8 tiles] so both
# cores of a pair run an IDENTICAL program (SPMD) with all per-core variation
# carried by input data (token permutation + boundary-mask patterns).
# Attention (softmax over the QUERY axis -> per-key normalizers Z[s]) is
# computed in S^T layout [s_partition, t_free], s-tile-major: one wide
# [128, 1024] PSUM strip per (pair, head, s_tile) and a single exp whose
# accum_out yields that s_tile's Z partial directly. Each core computes exp
# only over its owned-query half; the pair's Z partials are combined with a
# per-pair AllGather (cheaper than AllReduce in latency) + local add.
# Program order: all pairs' pass-1 + collectives are issued before any
# pass-2 so the collectives overlap pass-1 of later pairs.
import sys

if "/opt/trn_rl_repo" not in sys.path:
    sys.path.insert(0, "/opt/trn_rl_repo")

import numpy as np
import ml_dtypes

BF16 = ml_dtypes.bfloat16

B, T, C, H, HS = 4, 2048, 384, 6, 64
D4 = 4 * C  # 1536
EPS = 1e-5
NPAIR = H // 2  # 3 head-pairs
P = 128
NT = T // P  # 16 token tiles
CH = 512
OWN = T // 2  # 1024 owned tokens per core
NEG = -30.0
SCALE = float(C) ** -0.5
N_CORES = 8
GROUPS = [[0, 1], [2, 3], [4, 5], [6, 7]]

_PROG_CACHE = {}


def _win(u):
    """Owned-t window of local s_tile u: (t_lo, width, mask_kind).

    Local layout: t-tiles 0..7 owned (phys 2u+par), s-tiles 0..7 owned,
    8..15 other (phys 2k+(1-par)). Owned s_tile u: valid t >= u*128 with a
    true tril diag at t-tile u. Other s_tile 8+k: valid t >= k*128 with an
    all-or-nothing boundary block at t-tile k (q3m: ones iff par==0).
    """
    if u < 8:
        return u * P, OWN - u * P, "tril"
    k = u - 8
    return k * P, OWN - k * P, "q3m"


def _pt_layout():
    """Column offset of each s_tile's strip inside ptt, per (pair, head)."""
    off = {}
    pos = 0
    for u in range(16):
        _, w, _ = _win(u)
        off[u] = pos
        pos += w
    return off, pos  # pos = 9216


def _build_program(nz, pt_fp8=False):
    """nz: dict of nonzero-bias flags (bqk, bv, bproj, b2)."""
    import concourse.bass as bass
    import concourse.bacc as bacc
    import concourse.mybir as mybir
    from concourse.tile import TileContext
    from contextlib import ExitStack

    f32 = mybir.dt.float32
    bf16 = mybir.dt.bfloat16
    pt_dt = mybir.dt.float8e4 if pt_fp8 else bf16
    AF = mybir.ActivationFunctionType
    ALU = mybir.AluOpType

    nc = bacc.Bacc("TRN2", target_bir_lowering=False)

    # x shipped partition-major ([P, tile, C]) so each quarter is ONE fat
    # DMA (3KB/partition contiguous) instead of 16 trickling 768B-descriptor
    # tile DMAs (which starved LN1 until ~40us on the baseline trace).
    xw_d = nc.dram_tensor("x_wide", [P, NT, C], bf16, kind="ExternalInput")
    wqk_d = nc.dram_tensor("wqk", [P, 3, NPAIR, 2, P], bf16, kind="ExternalInput")
    wv_d = nc.dram_tensor("wv", [P, 3, C], bf16, kind="ExternalInput")
    wp_d = nc.dram_tensor("wp", [P, 3, C], bf16, kind="ExternalInput")
    w1_d = nc.dram_tensor("w1", [P, 3, D4], bf16, kind="ExternalInput")
    w2_d = nc.dram_tensor("w2", [P, 12, C], bf16, kind="ExternalInput")
    b1_d = nc.dram_tensor("b1", [P, 12], f32, kind="ExternalInput")
    ident_d = nc.dram_tensor("ident", [P, P], bf16, kind="ExternalInput")
    negi_d = nc.dram_tensor("negi", [P, P], bf16, kind="ExternalInput")
    tril_d = nc.dram_tensor("trilm", [P, P], bf16, kind="ExternalInput")
    q3m_d = nc.dram_tensor("q3m", [P, P], bf16, kind="ExternalInput")
    swapsel_d = nc.dram_tensor("swapsel", [P, 16], mybir.dt.uint8,
                               kind="ExternalInput")
    if nz["bqk"]:
        bqk_d = nc.dram_tensor("bqk", [P, NPAIR, 2], f32, kind="ExternalInput")
    if nz["bv"]:
        bv_d = nc.dram_tensor("bv", [P, C], f32, kind="ExternalInput")
    if nz["bproj"]:
        bproj_d = nc.dram_tensor("bproj", [P, C], f32, kind="ExternalInput")
    if nz["b2"]:
        b2_d = nc.dram_tensor("b2", [P, C], f32, kind="ExternalInput")
    zin_d = nc.dram_tensor("zin", [NPAIR, 2, P, 16], f32)
    zout_d = nc.dram_tensor("zout", [NPAIR, 2, 2, P, 16], f32)
    out_d = nc.dram_tensor("out", [OWN, C], f32, kind="ExternalOutput")

    pt_off, pt_cols = _pt_layout()

    with TileContext(nc) as tc, ExitStack() as ctx:
        cst = ctx.enter_context(tc.tile_pool(name="const", bufs=1))
        persist = ctx.enter_context(tc.tile_pool(name="persist", bufs=1))
        lnp = ctx.enter_context(tc.tile_pool(name="ln", bufs=4))
        qkp = ctx.enter_context(tc.tile_pool(name="qk", bufs=2))
        ptp = ctx.enter_context(
            tc.tile_pool(name="ptp", bufs=(6 if pt_fp8 else 4)))
        zp = ctx.enter_context(tc.tile_pool(name="zp", bufs=3))
        hidp = ctx.enter_context(tc.tile_pool(name="hid", bufs=7))
        outp = ctx.enter_context(tc.tile_pool(name="outp", bufs=3))
        ps_s = ctx.enter_context(tc.tile_pool(name="ps_s", bufs=2, space="PSUM"))
        ps_mm = ctx.enter_context(tc.tile_pool(name="ps_mm", bufs=2, space="PSUM"))
        ps_ab = ctx.enter_context(tc.tile_pool(name="ps_ab", bufs=2, space="PSUM"))

        # ---- constants / weights. ident + the fat x DMAs go on the SP
        # HWDGE queue (compute can start ~2us in); everything else via the
        # otherwise-idle gpsimd SWDGE queue, ordered by first use.
        ident_sb = cst.tile([P, P], bf16)
        nc.sync.dma_start(out=ident_sb, in_=ident_d[:])
        x_sb = persist.tile([P, NT, C], bf16)
        for qtr in range(4):
            nc.sync.dma_start(out=x_sb[:, qtr * 4:(qtr + 1) * 4, :],
                              in_=xw_d[:, qtr * 4:(qtr + 1) * 4, :])
        wv_sb = cst.tile([P, 3, C], bf16)
        nc.gpsimd.dma_start(out=wv_sb, in_=wv_d[:])
        wqk_sb = cst.tile([P, 3, NPAIR, 2, P], bf16)
        nc.gpsimd.dma_start(out=wqk_sb, in_=wqk_d[:])
        negi_sb = cst.tile([P, P], bf16)
        nc.gpsimd.dma_start(out=negi_sb, in_=negi_d[:])
        mask_sb = {}
        for nm, d in (("tril", tril_d), ("q3m", q3m_d)):
            m = cst.tile([P, P], bf16, name=f"m_{nm}")
            nc.gpsimd.dma_start(out=m, in_=d[:])
            mask_sb[nm] = m
        swapsel_sb = cst.tile([P, 16], mybir.dt.uint8)
        nc.gpsimd.dma_start(out=swapsel_sb, in_=swapsel_d[:])
        if nz["bqk"]:
            bqk_sb = cst.tile([P, NPAIR, 2], f32)
            nc.gpsimd.dma_start(out=bqk_sb, in_=bqk_d[:])
        if nz["bv"]:
            bv_sb = cst.tile([P, C], f32)
            nc.gpsimd.dma_start(out=bv_sb, in_=bv_d[:])
        # tail-stage weights: tiles declared here, DMAs issued later (inside
        # the attention pipeline) so they don't crowd x/wv/wqk at startup
        wp_sb = cst.tile([P, 3, C], bf16)
        b1_sb = cst.tile([P, 12], f32)
        w1_sb = cst.tile([P, 3, D4], bf16)
        w2_sb = cst.tile([P, 12, C], bf16)
        if nz["bproj"]:
            bproj_sb = cst.tile([P, C], f32)
        if nz["b2"]:
            b2_sb = cst.tile([P, C], f32)

        def late_weight_dmas(stage):
            if stage == 0:
                nc.gpsimd.dma_start(out=wp_sb, in_=wp_d[:])
                nc.gpsimd.dma_start(out=b1_sb, in_=b1_d[:])
                if nz["bproj"]:
                    nc.gpsimd.dma_start(out=bproj_sb, in_=bproj_d[:])
                if nz["b2"]:
                    nc.gpsimd.dma_start(out=b2_sb, in_=b2_d[:])
            else:
                nc.gpsimd.dma_start(out=w1_sb, in_=w1_d[:])
                nc.gpsimd.dma_start(out=w2_sb, in_=w2_d[:])

        eps_sb = cst.tile([P, 1], f32)
        nc.vector.memset(eps_sb, EPS)

        hT = persist.tile([P, 3, T], bf16)       # normalized x, transposed
        v_sb = persist.tile([P, NT, C], bf16)    # V (later scaled to V/Z)
        att_sb = persist.tile([P, NPAIR, OWN], bf16)  # attention out^T
        h2T = persist.tile([P, 3, OWN], bf16)    # LN2 out, transposed
        r_sb = persist.tile([P, 8, C], f32)      # residual-1 tiles (owned)

        # ---- LN1 + transpose into hT (x arrives in 4 fat quarter-DMAs).
        # Sqrt is batched over 8 tiles' variances at a time: every Sqrt is
        # then dependency-ordered BEFORE the first pass-1 Exp, so the ACT
        # table (Sqrt and Exp live in different sets) loads exactly once
        # per phase instead of toggling 1.3us reloads mid-exp-stream.
        # LN1 reads a host-provided bf16 copy of x (half the DMA bytes, 2x
        # DVE modes); the residual path uses the f32 owned-half copy.
        xts = [x_sb[:, i, :] for i in range(NT)]
        mvs = persist.tile([P, NT, 2], f32)
        rs16 = persist.tile([P, NT], f32)
        groups = [(0, 2), (2, 4), (4, 8), (8, 12), (12, 16)]
        for lo, hi in groups:
            for i in range(lo, hi):
                st = lnp.tile([P, 6], f32, name="st")
                nc.vector.bn_stats(out=st, in_=xts[i])
                nc.vector.bn_aggr(out=mvs[:, i, :], in_=st)
            nc.scalar.activation(out=rs16[:, lo:hi],
                                 in_=mvs[:, lo:hi, 1],
                                 func=AF.Sqrt, bias=eps_sb)
            nc.vector.reciprocal(out=rs16[:, lo:hi], in_=rs16[:, lo:hi])
            for i in range(lo, hi):
                hb = lnp.tile([P, C], bf16, name="hb")
                nc.vector.tensor_scalar(out=hb, in0=xts[i],
                                        scalar1=mvs[:, i, 0:1],
                                        scalar2=rs16[:, i:i + 1],
                                        op0=ALU.subtract, op1=ALU.mult)
                for cc in range(3):
                    tp = ps_ab.tile([P, P], bf16, name="tp", tag="ab")
                    nc.tensor.transpose(tp, hb[:, cc * P:(cc + 1) * P],
                                        ident_sb)
                    # let the scheduler balance these across ACT/DVE
                    nc.any.tensor_copy(hT[:, cc, i * P:(i + 1) * P], tp)

        def v_loop():
            # V for all heads (lhsT = hT chunk, rhs = wv). Emitted after
            # QK(0)+pass-1(0) so it fills PE/DVE idle time while ACT runs
            # pair 0's exps (V isn't read until pass-2 of pair 0).
            for i in range(NT):
                pv = ps_mm.tile([P, C], f32, name="pv", tag="pq")
                for cc in range(3):
                    nc.tensor.matmul(pv, hT[:, cc, i * P:(i + 1) * P],
                                     wv_sb[:, cc, :], start=(cc == 0),
                                     stop=(cc == 2))
                if nz["bv"]:
                    nc.vector.tensor_add(out=v_sb[:, i, :], in0=pv, in1=bv_sb)
                else:
                    nc.vector.tensor_copy(v_sb[:, i, :], pv)

        # ---- attention, software-pipelined per head-pair:
        #   QK(p) -> pass-1(p) -> Z AllGather(p) issued, then while it (and
        #   the next pair's pass-1) runs: Z-combine(p-1) + pass-2(p-1).
        # ptt slot rotation (bufs=4) matches this program order.
        qt = {}
        kt = {}
        ptt = {}

        def qk_pass1(p):
            qt[p] = qkp.tile([P, OWN], bf16, name=f"qt{p}", tag="qt")
            kt[p] = qkp.tile([P, T], bf16, name=f"kt{p}", tag="kt")
            # qt first, then kt chunks in order: the s_tile-0 exp only needs
            # qt + kt chunk 0, so the first exp can start ASAP
            for qk, dst, nch in ((0, qt[p], 2), (1, kt[p], 4)):
                for c in range(nch):
                    pq = ps_mm.tile([P, CH], f32, name="pq")
                    for cc in range(3):
                        nc.tensor.matmul(pq, wqk_sb[:, cc, p, qk, :],
                                         hT[:, cc, c * CH:(c + 1) * CH],
                                         start=(cc == 0), stop=(cc == 2))
                    if nz["bqk"]:
                        nc.vector.tensor_scalar(
                            out=dst[:, c * CH:(c + 1) * CH], in0=pq,
                            scalar1=bqk_sb[:, p, qk:qk + 1], scalar2=None,
                            op0=ALU.add)
                    else:
                        nc.vector.tensor_copy(dst[:, c * CH:(c + 1) * CH], pq)

            for h in range(2):
                ptt[(p, h)] = ptp.tile([P, pt_cols], pt_dt,
                                       name=f"pt{p}_{h}", tag="pt")
                zl = zp.tile([P, 16], f32, name=f"zl{p}_{h}", tag="zl", bufs=6)
                hb_ = h * 64
                for u in range(16):
                    tl, w, mk = _win(u)
                    sp = ps_s.tile([P, 1024], f32, name="sp", tag="sp")
                    nmm = (w + CH - 1) // CH
                    for j in range(nmm):
                        wj = min(CH, w - j * CH)
                        nc.tensor.matmul(
                            sp[:, j * CH:j * CH + wj],
                            kt[p][hb_:hb_ + 64, u * P:(u + 1) * P],
                            qt[p][hb_:hb_ + 64, tl + j * CH:tl + j * CH + wj],
                            start=True, stop=(j > 0))
                        if j == 0:
                            # -30 on the masked part of the boundary/diag
                            # tile (first 128 cols), accumulated via PE.
                            nc.tensor.matmul(sp[:, 0:P], negi_sb, mask_sb[mk],
                                             start=False, stop=True)
                    # no accum_out: the 96 ACTIVATION_READ_ACCUMULATOR
                    # reads cost ~27us of serial ACT time (the pacing
                    # engine); Z comes from a DVE reduce of the bf16
                    # strip instead (consistent with what pass-2 sums).
                    nc.scalar.activation(
                        out=ptt[(p, h)][:, pt_off[u]:pt_off[u] + w],
                        in_=sp[:, :w], func=AF.Exp)
                    nc.vector.reduce_sum(
                        out=zl[:, u:u + 1],
                        in_=ptt[(p, h)][:, pt_off[u]:pt_off[u] + w],
                        axis=mybir.AxisListType.X)
                nc.sync.dma_start(out=zin_d[p, h], in_=zl)
            nc.gpsimd.collective_compute(
                "AllGather", ALU.bypass, replica_groups=GROUPS,
                ins=[zin_d[p]], outs=[zout_d[p]])

        zgs = {}

        def zg_fetch(p):
            # issued early so these DMAs sit ahead of the NEXT pair's zin
            # on the in-order Pool queue (they only wait on collective p)
            zg = zp.tile([P, 2, 2, 16], f32, name=f"zg{p}", tag="zg")
            nc.gpsimd.dma_start(
                out=zg, in_=zout_d[p].rearrange("r h p z -> p r h z"))
            zgs[p] = zg

        def zfix(p):
            # combine Z partials from the AllGather, scale V cols by 1/Z
            zg = zgs[p]
            for h in range(2):
                # Z_local = mine + swap8(partner). With g0/g1 in replica
                # order and A = g0 + swap8(g1): par==0 -> A, par==1 ->
                # swap8(A); selected via the swapsel input (==par).
                za = zp.tile([P, 16], f32, name=f"za{p}_{h}", tag="za")
                nc.vector.tensor_tensor(out=za[:, 0:8], in0=zg[:, 0, h, 0:8],
                                        in1=zg[:, 1, h, 8:16], op=ALU.add)
                nc.vector.tensor_tensor(out=za[:, 8:16], in0=zg[:, 0, h, 8:16],
                                        in1=zg[:, 1, h, 0:8], op=ALU.add)
                zb = zp.tile([P, 16], f32, name=f"zb{p}_{h}", tag="zb")
                nc.vector.tensor_copy(zb[:, 0:8], za[:, 8:16])
                nc.vector.tensor_copy(zb[:, 8:16], za[:, 0:8])
                nc.vector.copy_predicated(za, swapsel_sb, zb)
                nc.vector.reciprocal(out=za, in_=za)
                col = (2 * p + h) * 64
                for k in range(16):
                    nc.vector.tensor_scalar_mul(
                        out=v_sb[:, k, col:col + 64],
                        in0=v_sb[:, k, col:col + 64], scalar1=za[:, k:k + 1])

        def pass2(p, c):
            # out^T chunk = sum_s (V/Z)^T-slices @ P^T
            pvp = ps_ab.tile([P, CH], f32, name="pvp", tag="ab")
            # heads interleaved: consecutive MMs target disjoint col groups
            # (tile_position 0 / 64) so the PE runs both heads' chains
            # CONCURRENTLY (col-tiling). PSUM started-state is tracked per
            # partition x zero-region, and the chains are partition-disjoint,
            # so interleaved start/stop flags are safe.
            us = [u for u in range(16) if _win(u)[0] < (c + 1) * CH]
            for n, u in enumerate(us):
                tl, w, _ = _win(u)
                lo = max(tl, c * CH)
                wid = (c + 1) * CH - lo
                for h in range(2):
                    nc.tensor.matmul(
                        pvp[h * 64:(h + 1) * 64, lo - c * CH:],
                        v_sb[:, u, (2 * p + h) * 64:(2 * p + h + 1) * 64],
                        ptt[(p, h)][:, pt_off[u] + lo - tl:
                                    pt_off[u] + lo - tl + wid],
                        start=(n == 0), stop=(n == len(us) - 1),
                        tile_position=(0, h * 64))
            nc.vector.tensor_copy(att_sb[:, p, c * CH:(c + 1) * CH], pvp)

        for p in range(NPAIR):
            if p >= 1:
                zg_fetch(p - 1)
            qk_pass1(p)
            if p == 0:
                v_loop()
            late_weight_dmas(p)
            if p >= 1:
                zfix(p - 1)
                pass2(p - 1, 0)
                pass2(p - 1, 1)
        zg_fetch(NPAIR - 1)
        zfix(NPAIR - 1)

        # ---- tail: last pair's pass-2 chunk-wise, proj + residual 1 + LN2
        # per 512-token chunk, then FFN per chunk.
        for c in range(2):
            pass2(NPAIR - 1, c)
            for i in range(c * 4, c * 4 + 4):
                py = ps_mm.tile([P, C], f32, name="py", tag="pq")
                for p in range(NPAIR):
                    nc.tensor.matmul(py, att_sb[:, p, i * P:(i + 1) * P],
                                     wp_sb[:, p, :], start=(p == 0), stop=(p == 2))
                # residual uses the bf16 x copy (x's bf16 quantization adds
                # ~0.3% rel err vs the 2e-2 gate; saves 12KB SBUF + a DMA)
                nc.vector.tensor_add(out=r_sb[:, i, :], in0=py,
                                     in1=x_sb[:, i, :])
                if nz["bproj"]:
                    nc.vector.tensor_add(out=r_sb[:, i, :], in0=r_sb[:, i, :],
                                         in1=bproj_sb)
                st2 = lnp.tile([P, 6], f32, name="st2")
                nc.vector.bn_stats(out=st2, in_=r_sb[:, i, :])
                mv2 = lnp.tile([P, 2], f32, name="mv2")
                nc.vector.bn_aggr(out=mv2, in_=st2)
                rs2 = lnp.tile([P, 1], f32, name="rs2")
                nc.scalar.activation(out=rs2, in_=mv2[:, 1:2], func=AF.Sqrt,
                                     bias=eps_sb)
                nc.vector.reciprocal(out=rs2, in_=rs2)
                h2b = lnp.tile([P, C], bf16, name="h2b")
                nc.vector.tensor_scalar(out=h2b, in0=r_sb[:, i, :],
                                        scalar1=mv2[:, 0:1], scalar2=rs2,
                                        op0=ALU.subtract, op1=ALU.mult)
                for cc in range(3):
                    tp2 = ps_ab.tile([P, P], bf16, name="tp2", tag="ab")
                    nc.tensor.transpose(tp2, h2b[:, cc * P:(cc + 1) * P],
                                        ident_sb)
                    nc.vector.tensor_copy(h2T[:, cc, i * P:(i + 1) * P], tp2)

        # ---- FFN + residual 2 + store. Two 512-wide hidden chunks share a
        # [P,1024] PSUM tile (ps_s is idle by now) and, when b1 is zero, a
        # single relu — halving the FFN1 matmul->relu round-trips.
        for c in range(2):
            hid = []
            for cb2 in range(6):
                ph = ps_s.tile([P, 1024], f32, name="ph", tag="sp")
                for half in range(2):
                    cb = 2 * cb2 + half
                    for cc in range(3):
                        nc.tensor.matmul(
                            ph[:, half * CH:(half + 1) * CH],
                            w1_sb[:, cc, cb * P:(cb + 1) * P],
                            h2T[:, cc, c * CH:(c + 1) * CH],
                            start=(cc == 0), stop=(cc == 2))
                ht_ = hidp.tile([P, 2, CH], bf16, name=f"ht{c}_{cb2}",
                                tag="hid")
                if nz["b1"]:
                    for half in range(2):
                        cb = 2 * cb2 + half
                        nc.scalar.activation(
                            out=ht_[:, half, :],
                            in_=ph[:, half * CH:(half + 1) * CH],
                            func=AF.Relu, bias=b1_sb[:, cb:cb + 1])
                else:
                    nc.scalar.activation(out=ht_, in_=ph, func=AF.Relu)
                hid.append(ht_)
            for jj in range(4):
                i = c * 4 + jj
                pf = ps_mm.tile([P, C], f32, name="pf", tag="pq")
                for cb in range(12):
                    nc.tensor.matmul(pf, hid[cb // 2][:, cb % 2,
                                                      jj * P:(jj + 1) * P],
                                     w2_sb[:, cb, :], start=(cb == 0),
                                     stop=(cb == 11))
                ot = outp.tile([P, C], f32, name="ot")
                nc.vector.tensor_add(out=ot, in0=pf, in1=r_sb[:, i, :])
                if nz["b2"]:
                    nc.vector.tensor_add(out=ot, in0=ot, in1=b2_sb)
                nc.sync.dma_start(out=out_d[i * P:(i + 1) * P, :], in_=ot)

    nc.compile()
    return nc


def _prep_inputs(inputs):
    """Host-side: fold gains into weights, build per-core input maps."""
    x = np.asarray(inputs["x"], np.float32)
    g1 = np.asarray(inputs["g1"], np.float32)
    be1 = np.asarray(inputs["be1"], np.float32)
    g2 = np.asarray(inputs["g2"], np.float32)
    be2 = np.asarray(inputs["be2"], np.float32)
    # attention scale folded into wq so masks added to S psum stay at NEG
    wq = np.asarray(inputs["wq"], np.float32) * g1[None, :, None] * SCALE
    wk = np.asarray(inputs["wk"], np.float32) * g1[None, :, None]
    wv = np.asarray(inputs["wv"], np.float32) * g1[None, :, None]
    bq = np.einsum("c,hcd->hd", be1,
                   np.asarray(inputs["wq"], np.float32)) * SCALE
    bk = np.einsum("c,hcd->hd", be1, np.asarray(inputs["wk"], np.float32))
    bv = np.einsum("c,hcd->hd", be1, np.asarray(inputs["wv"], np.float32))
    wp = np.asarray(inputs["w_proj"], np.float32)
    bproj = np.asarray(inputs["b_proj"], np.float32)
    w1 = np.asarray(inputs["w1"], np.float32) * g2[:, None]
    b1 = np.asarray(inputs["b1"], np.float32) + be2 @ np.asarray(
        inputs["w1"], np.float32)
    w2 = np.asarray(inputs["w2"], np.float32)
    b2 = np.asarray(inputs["b2"], np.float32)

    nz = dict(bqk=bool(np.any(bq) or np.any(bk)), bv=bool(np.any(bv)),
              bproj=bool(np.any(bproj)), b2=bool(np.any(b2)),
              b1=bool(np.any(b1)))

    # wqk [128, cc, pair, qk, col]: lhsT chunks (c-partition, head-pair cols)
    wqk = np.zeros((P, 3, NPAIR, 2, P), BF16)
    for pr in range(NPAIR):
        for qk, w in ((0, wq), (1, wk)):
            pair = np.concatenate([w[2 * pr], w[2 * pr + 1]], axis=1)  # [C,128]
            wqk[:, :, pr, qk, :] = pair.reshape(3, P, P).transpose(1, 0, 2)
    wv_all = np.concatenate([wv[h] for h in range(H)], axis=1)  # [C, 384]
    wv_pre = wv_all.reshape(3, P, C).transpose(1, 0, 2).astype(BF16)
    wp_pre = wp.reshape(3, P, C).transpose(1, 0, 2).astype(BF16)
    w1_pre = w1.reshape(3, P, D4).transpose(1, 0, 2).astype(BF16)
    w2_pre = w2.reshape(12, P, C).transpose(1, 0, 2).astype(BF16)
    b1_pre = np.ascontiguousarray(b1.reshape(12, P).T).astype(np.float32)

    ident = np.eye(P, dtype=BF16)
    negi = (np.eye(P) * NEG).astype(BF16)
    sl = np.tril(np.ones((P, P)), -1).astype(BF16)  # strict lower: s > t

    common = dict(wqk=wqk, wv=wv_pre, wp=wp_pre, w1=w1_pre, w2=w2_pre,
                  b1=b1_pre, ident=ident, negi=negi, trilm=sl)
    if nz["bqk"]:
        bqk = np.zeros((P, NPAIR, 2), np.float32)
        for pr in range(NPAIR):
            bqk[:, pr, 0] = np.concatenate([bq[2 * pr], bq[2 * pr + 1]])
            bqk[:, pr, 1] = np.concatenate([bk[2 * pr], bk[2 * pr + 1]])
        common["bqk"] = bqk
    if nz["bv"]:
        common["bv"] = np.broadcast_to(
            np.concatenate([bv[h] for h in range(H)]), (P, C)).copy()
    if nz["bproj"]:
        common["bproj"] = np.broadcast_to(bproj, (P, C)).copy()
    if nz["b2"]:
        common["b2"] = np.broadcast_to(b2, (P, C)).copy()

    ones = np.ones((P, P), BF16)
    zeros = np.zeros((P, P), BF16)
    in_maps = []
    for core in range(N_CORES):
        b, par = core // 2, core % 2
        perm = list(range(par, NT, 2)) + list(range(1 - par, NT, 2))
        xt = x[b].reshape(NT, P, C)[perm]  # [NT, P, C]
        m = dict(common)
        m["x_wide"] = np.ascontiguousarray(
            xt.transpose(1, 0, 2)).astype(BF16)
        # q3 boundary (s other, t owned): phys 2u+(1-par) vs 2u+par:
        #   par=0: s odd > t even at boundary -> invalid -> mask ON
        m["q3m"] = ones if par == 0 else zeros
        m["swapsel"] = np.full((P, 16), par, np.uint8)
        in_maps.append(m)
    return in_maps, nz


def _purge_neff_cache():
    # libneuronxla's NEFF cache is keyed on the HLO module hash, which does
    # not cover the BIR carried in backend_config -- a stale kernel body can
    # be silently reused across program edits. Purge before compiling.
    import glob, os, shutil
    for d in glob.glob(os.path.expanduser(
            "~/.neuron-compile-cache/*/MODULE_*")):
        try:
            shutil.rmtree(d, ignore_errors=True)
        except OSError:
            pass


def kernel(**inputs):
    import os
    pt_fp8 = os.environ.get("KPT8", "0") == "1"
    _purge_neff_cache()
    in_maps, nz = _prep_inputs(inputs)
    key = (pt_fp8, tuple(sorted(nz.items())))
    if key not in _PROG_CACHE:
        _PROG_CACHE[key] = _build_program(nz, pt_fp8)
    nc = _PROG_CACHE[key]

    from concourse.bass_utils import run_bass_kernel_spmd
    res = run_bass_kernel_spmd(nc, in_maps, core_ids=list(range(N_CORES)))

    out = np.empty((B, T, C), np.float32)
    for core in range(N_CORES):
        b, par = core // 2, core % 2
        tiles = out[b].reshape(NT, P, C)
        tiles[par::2] = res.results[core]["out"].reshape(8, P, C)
    return out



# revision 14
# speedup vs baseline: 1.0541x; 1.0541x over previous
# Trainium2 Bass kernel for nn_Block_9483287789889 (dense transformer block).
#
# Sharding (8 cores): 2 cores per batch (B=4). Host permutes each batch's
# 2048 tokens into [owned 8x128-tiles (interleaved) | other 8 tiles] so both
# cores of a pair run an IDENTICAL program (SPMD) with all per-core variation
# carried by input data (token permutation + boundary-mask patterns).
# Attention (softmax over the QUERY axis -> per-key normalizers Z[s]) is
# computed in S^T layout [s_partition, t_free], s-tile-major: one wide
# [128, 1024] PSUM strip per (pair, head, s_tile) and a single exp whose
# accum_out yields that s_tile's Z partial directly. Each core computes exp
# only over its owned-query half; the pair's Z partials are combined with a
# per-pair AllGather (cheaper than AllReduce in latency) + local add.
# Program order: all pairs' pass-1 + collectives are issued before any
# pass-2 so the collectives overlap pass-1 of later pairs.
import sys

if "/opt/trn_rl_repo" not in sys.path:
    sys.path.insert(0, "/opt/trn_rl_repo")

import numpy as np
import ml_dtypes

BF16 = ml_dtypes.bfloat16

B, T, C, H, HS = 4, 2048, 384, 6, 64
D4 = 4 * C  # 1536
EPS = 1e-5
NPAIR = H // 2  # 3 head-pairs
P = 128
NT = T // P  # 16 token tiles
CH = 512
OWN = T // 2  # 1024 owned tokens per core
NEG = -30.0
SCALE = float(C) ** -0.5
N_CORES = 8
GROUPS = [[0, 1], [2, 3], [4, 5], [6, 7]]

_PROG_CACHE = {}


def _win(u):
    """Owned-t window of local s_tile u: (t_lo, width, mask_kind).

    Local layout: t-tiles 0..7 owned (phys 2u+par), s-tiles 0..7 owned,
    8..15 other (phys 2k+(1-par)). Owned s_tile u: valid t >= u*128 with a
    true tril diag at t-tile u. Other s_tile 8+k: valid t >= k*128 with an
    all-or-nothing boundary block at t-tile k (q3m: ones iff par==0).
    """
    if u < 8:
        return u * P, OWN - u * P, "tril"
    k = u - 8
    return k * P, OWN - k * P, "q3m"


def _pt_layout():
    """Column offset of each s_tile's strip inside ptt, per (pair, head)."""
    off = {}
    pos = 0
    for u in range(16):
        _, w, _ = _win(u)
        off[u] = pos
        pos += w
    return off, pos  # pos = 9216


def _build_program(nz, pt_fp8=False):
    """nz: dict of nonzero-bias flags (bqk, bv, bproj, b2)."""
    import concourse.bass as bass
    import concourse.bacc as bacc
    import concourse.mybir as mybir
    from concourse.tile import TileContext
    from contextlib import ExitStack

    f32 = mybir.dt.float32
    bf16 = mybir.dt.bfloat16
    pt_dt = mybir.dt.float8e4 if pt_fp8 else bf16
    AF = mybir.ActivationFunctionType
    ALU = mybir.AluOpType

    nc = bacc.Bacc("TRN2", target_bir_lowering=False)

    # x shipped partition-major ([P, tile, C]) so each quarter is ONE fat
    # DMA (3KB/partition contiguous) instead of 16 trickling 768B-descriptor
    # tile DMAs (which starved LN1 until ~40us on the baseline trace).
    xw_d = nc.dram_tensor("x_wide", [P, NT, C], bf16, kind="ExternalInput")
    wqk_d = nc.dram_tensor("wqk", [P, 3, NPAIR, 2, P], bf16, kind="ExternalInput")
    wv_d = nc.dram_tensor("wv", [P, 3, C], bf16, kind="ExternalInput")
    wp_d = nc.dram_tensor("wp", [P, 3, C], bf16, kind="ExternalInput")
    w1_d = nc.dram_tensor("w1", [P, 3, D4], bf16, kind="ExternalInput")
    w2_d = nc.dram_tensor("w2", [P, 12, C], bf16, kind="ExternalInput")
    b1_d = nc.dram_tensor("b1", [P, 12], f32, kind="ExternalInput")
    ident_d = nc.dram_tensor("ident", [P, P], bf16, kind="ExternalInput")
    negi_d = nc.dram_tensor("negi", [P, P], bf16, kind="ExternalInput")
    tril_d = nc.dram_tensor("trilm", [P, P], bf16, kind="ExternalInput")
    q3m_d = nc.dram_tensor("q3m", [P, P], bf16, kind="ExternalInput")
    swapsel_d = nc.dram_tensor("swapsel", [P, 16], mybir.dt.uint8,
                               kind="ExternalInput")
    if nz["bqk"]:
        bqk_d = nc.dram_tensor("bqk", [P, NPAIR, 2], f32, kind="ExternalInput")
    if nz["bv"]:
        bv_d = nc.dram_tensor("bv", [P, C], f32, kind="ExternalInput")
    if nz["bproj"]:
        bproj_d = nc.dram_tensor("bproj", [P, C], f32, kind="ExternalInput")
    if nz["b2"]:
        b2_d = nc.dram_tensor("b2", [P, C], f32, kind="ExternalInput")
    zin_d = nc.dram_tensor("zin", [NPAIR, 2, P, 16], f32)
    zout_d = nc.dram_tensor("zout", [NPAIR, 2, 2, P, 16], f32)
    out_d = nc.dram_tensor("out", [OWN, C], f32, kind="ExternalOutput")

    pt_off, pt_cols = _pt_layout()

    with TileContext(nc) as tc, ExitStack() as ctx:
        cst = ctx.enter_context(tc.tile_pool(name="const", bufs=1))
        persist = ctx.enter_context(tc.tile_pool(name="persist", bufs=1))
        lnp = ctx.enter_context(tc.tile_pool(name="ln", bufs=4))
        qkp = ctx.enter_context(tc.tile_pool(name="qk", bufs=2))
        ptp = ctx.enter_context(
            tc.tile_pool(name="ptp", bufs=(6 if pt_fp8 else 4)))
        zp = ctx.enter_context(tc.tile_pool(name="zp", bufs=3))
        hidp = ctx.enter_context(tc.tile_pool(name="hid", bufs=7))
        outp = ctx.enter_context(tc.tile_pool(name="outp", bufs=3))
        ps_s = ctx.enter_context(tc.tile_pool(name="ps_s", bufs=2, space="PSUM"))
        ps_mm = ctx.enter_context(tc.tile_pool(name="ps_mm", bufs=2, space="PSUM"))
        ps_ab = ctx.enter_context(tc.tile_pool(name="ps_ab", bufs=2, space="PSUM"))

        # ---- constants / weights. ident + the fat x DMAs go on the SP
        # HWDGE queue (compute can start ~2us in); everything else via the
        # otherwise-idle gpsimd SWDGE queue, ordered by first use.
        ident_sb = cst.tile([P, P], bf16)
        nc.sync.dma_start(out=ident_sb, in_=ident_d[:])
        x_sb = persist.tile([P, NT, C], bf16)
        for qtr in range(4):
            nc.sync.dma_start(out=x_sb[:, qtr * 4:(qtr + 1) * 4, :],
                              in_=xw_d[:, qtr * 4:(qtr + 1) * 4, :])
        wv_sb = cst.tile([P, 3, C], bf16)
        nc.gpsimd.dma_start(out=wv_sb, in_=wv_d[:])
        wqk_sb = cst.tile([P, 3, NPAIR, 2, P], bf16)
        nc.gpsimd.dma_start(out=wqk_sb, in_=wqk_d[:])
        negi_sb = cst.tile([P, P], bf16)
        nc.gpsimd.dma_start(out=negi_sb, in_=negi_d[:])
        mask_sb = {}
        for nm, d in (("tril", tril_d), ("q3m", q3m_d)):
            m = cst.tile([P, P], bf16, name=f"m_{nm}")
            nc.gpsimd.dma_start(out=m, in_=d[:])
            mask_sb[nm] = m
        swapsel_sb = cst.tile([P, 16], mybir.dt.uint8)
        nc.gpsimd.dma_start(out=swapsel_sb, in_=swapsel_d[:])
        if nz["bqk"]:
            bqk_sb = cst.tile([P, NPAIR, 2], f32)
            nc.gpsimd.dma_start(out=bqk_sb, in_=bqk_d[:])
        if nz["bv"]:
            bv_sb = cst.tile([P, C], f32)
            nc.gpsimd.dma_start(out=bv_sb, in_=bv_d[:])
        # tail-stage weights: tiles declared here, DMAs issued later (inside
        # the attention pipeline) so they don't crowd x/wv/wqk at startup
        wp_sb = cst.tile([P, 3, C], bf16)
        b1_sb = cst.tile([P, 12], f32)
        w1_sb = cst.tile([P, 3, D4], bf16)
        w2_sb = cst.tile([P, 12, C], bf16)
        if nz["bproj"]:
            bproj_sb = cst.tile([P, C], f32)
        if nz["b2"]:
            b2_sb = cst.tile([P, C], f32)

        def late_weight_dmas(stage):
            if stage == 0:
                nc.gpsimd.dma_start(out=wp_sb, in_=wp_d[:])
                nc.gpsimd.dma_start(out=b1_sb, in_=b1_d[:])
                if nz["bproj"]:
                    nc.gpsimd.dma_start(out=bproj_sb, in_=bproj_d[:])
                if nz["b2"]:
                    nc.gpsimd.dma_start(out=b2_sb, in_=b2_d[:])
            else:
                nc.gpsimd.dma_start(out=w1_sb, in_=w1_d[:])
                nc.gpsimd.dma_start(out=w2_sb, in_=w2_d[:])

        eps_sb = cst.tile([P, 1], f32)
        nc.vector.memset(eps_sb, EPS)

        hT = persist.tile([P, 3, T], bf16)       # normalized x, transposed
        v_sb = persist.tile([P, NT, C], bf16)    # V (later scaled to V/Z)
        att_sb = persist.tile([P, NPAIR, OWN], bf16)  # attention out^T
        h2T = persist.tile([P, 3, OWN], bf16)    # LN2 out, transposed
        r_sb = persist.tile([P, 8, C], f32)      # residual-1 tiles (owned)

        # ---- LN1 + transpose into hT (x arrives in 4 fat quarter-DMAs).
        # Sqrt is batched over 8 tiles' variances at a time: every Sqrt is
        # then dependency-ordered BEFORE the first pass-1 Exp, so the ACT
        # table (Sqrt and Exp live in different sets) loads exactly once
        # per phase instead of toggling 1.3us reloads mid-exp-stream.
        # LN1 reads a host-provided bf16 copy of x (half the DMA bytes, 2x
        # DVE modes); the residual path uses the f32 owned-half copy.
        xts = [x_sb[:, i, :] for i in range(NT)]
        mvs = persist.tile([P, NT, 2], f32)
        rs16 = persist.tile([P, NT], f32)
        groups = [(0, 2), (2, 4), (4, 8), (8, 12), (12, 16)]
        for lo, hi in groups:
            for i in range(lo, hi):
                st = lnp.tile([P, 6], f32, name="st")
                nc.vector.bn_stats(out=st, in_=xts[i])
                nc.vector.bn_aggr(out=mvs[:, i, :], in_=st)
            nc.scalar.activation(out=rs16[:, lo:hi],
                                 in_=mvs[:, lo:hi, 1],
                                 func=AF.Sqrt, bias=eps_sb)
            nc.vector.reciprocal(out=rs16[:, lo:hi], in_=rs16[:, lo:hi])
            for i in range(lo, hi):
                hb = lnp.tile([P, C], bf16, name="hb")
                nc.vector.tensor_scalar(out=hb, in0=xts[i],
                                        scalar1=mvs[:, i, 0:1],
                                        scalar2=rs16[:, i:i + 1],
                                        op0=ALU.subtract, op1=ALU.mult)
                for cc in range(3):
                    tp = ps_ab.tile([P, P], bf16, name="tp", tag="ab")
                    nc.tensor.transpose(tp, hb[:, cc * P:(cc + 1) * P],
                                        ident_sb)
                    # let the scheduler balance these across ACT/DVE
                    nc.any.tensor_copy(hT[:, cc, i * P:(i + 1) * P], tp)

        def v_loop():
            # V for all heads (lhsT = hT chunk, rhs = wv). Emitted after
            # QK(0)+pass-1(0) so it fills PE/DVE idle time while ACT runs
            # pair 0's exps (V isn't read until pass-2 of pair 0).
            for i in range(NT):
                pv = ps_mm.tile([P, C], f32, name="pv", tag="pq")
                for cc in range(3):
                    nc.tensor.matmul(pv, hT[:, cc, i * P:(i + 1) * P],
                                     wv_sb[:, cc, :], start=(cc == 0),
                                     stop=(cc == 2))
                if nz["bv"]:
                    nc.vector.tensor_add(out=v_sb[:, i, :], in0=pv, in1=bv_sb)
                else:
                    nc.vector.tensor_copy(v_sb[:, i, :], pv)

        # ---- attention, software-pipelined per head-pair:
        #   QK(p) -> pass-1(p) -> Z AllGather(p) issued, then while it (and
        #   the next pair's pass-1) runs: Z-combine(p-1) + pass-2(p-1).
        # ptt slot rotation (bufs=4) matches this program order.
        qt = {}
        kt = {}
        ptt = {}

        def qk_pass1(p):
            qt[p] = qkp.tile([P, OWN], bf16, name=f"qt{p}", tag="qt")
            kt[p] = qkp.tile([P, T], bf16, name=f"kt{p}", tag="kt")
            # qt first, then kt chunks in order: the s_tile-0 exp only needs
            # qt + kt chunk 0, so the first exp can start ASAP
            for qk, dst, nch in ((0, qt[p], 2), (1, kt[p], 4)):
                for c in range(nch):
                    pq = ps_mm.tile([P, CH], f32, name="pq")
                    for cc in range(3):
                        nc.tensor.matmul(pq, wqk_sb[:, cc, p, qk, :],
                                         hT[:, cc, c * CH:(c + 1) * CH],
                                         start=(cc == 0), stop=(cc == 2))
                    if nz["bqk"]:
                        nc.vector.tensor_scalar(
                            out=dst[:, c * CH:(c + 1) * CH], in0=pq,
                            scalar1=bqk_sb[:, p, qk:qk + 1], scalar2=None,
                            op0=ALU.add)
                    else:
                        nc.vector.tensor_copy(dst[:, c * CH:(c + 1) * CH], pq)

            for h in range(2):
                ptt[(p, h)] = ptp.tile([P, pt_cols], pt_dt,
                                       name=f"pt{p}_{h}", tag="pt")
                zl = zp.tile([P, 16], f32, name=f"zl{p}_{h}", tag="zl", bufs=6)
                hb_ = h * 64
                for u in range(16):
                    tl, w, mk = _win(u)
                    sp = ps_s.tile([P, 1024], f32, name="sp", tag="sp")
                    nmm = (w + CH - 1) // CH
                    for j in range(nmm):
                        wj = min(CH, w - j * CH)
                        nc.tensor.matmul(
                            sp[:, j * CH:j * CH + wj],
                            kt[p][hb_:hb_ + 64, u * P:(u + 1) * P],
                            qt[p][hb_:hb_ + 64, tl + j * CH:tl + j * CH + wj],
                            start=True, stop=(j > 0))
                        if j == 0:
                            # -30 on the masked part of the boundary/diag
                            # tile (first 128 cols), accumulated via PE.
                            nc.tensor.matmul(sp[:, 0:P], negi_sb, mask_sb[mk],
                                             start=False, stop=True)
                    nc.scalar.activation(
                        out=ptt[(p, h)][:, pt_off[u]:pt_off[u] + w],
                        in_=sp[:, :w], func=AF.Exp,
                        accum_out=zl[:, u:u + 1])
                nc.sync.dma_start(out=zin_d[p, h], in_=zl)
            nc.gpsimd.collective_compute(
                "AllGather", ALU.bypass, replica_groups=GROUPS,
                ins=[zin_d[p]], outs=[zout_d[p]])

        zgs = {}

        def zg_fetch(p):
            # issued early so these DMAs sit ahead of the NEXT pair's zin
            # on the in-order Pool queue (they only wait on collective p)
            zg = zp.tile([P, 2, 2, 16], f32, name=f"zg{p}", tag="zg")
            nc.gpsimd.dma_start(
                out=zg, in_=zout_d[p].rearrange("r h p z -> p r h z"))
            zgs[p] = zg

        def zfix(p):
            # combine Z partials from the AllGather, scale V cols by 1/Z
            zg = zgs[p]
            for h in range(2):
                # Z_local = mine + swap8(partner). With g0/g1 in replica
                # order and A = g0 + swap8(g1): par==0 -> A, par==1 ->
                # swap8(A); selected via the swapsel input (==par).
                za = zp.tile([P, 16], f32, name=f"za{p}_{h}", tag="za")
                nc.vector.tensor_tensor(out=za[:, 0:8], in0=zg[:, 0, h, 0:8],
                                        in1=zg[:, 1, h, 8:16], op=ALU.add)
                nc.vector.tensor_tensor(out=za[:, 8:16], in0=zg[:, 0, h, 8:16],
                                        in1=zg[:, 1, h, 0:8], op=ALU.add)
                zb = zp.tile([P, 16], f32, name=f"zb{p}_{h}", tag="zb")
                nc.vector.tensor_copy(zb[:, 0:8], za[:, 8:16])
                nc.vector.tensor_copy(zb[:, 8:16], za[:, 0:8])
                nc.vector.copy_predicated(za, swapsel_sb, zb)
                nc.vector.reciprocal(out=za, in_=za)
                col = (2 * p + h) * 64
                for k in range(16):
                    nc.vector.tensor_scalar_mul(
                        out=v_sb[:, k, col:col + 64],
                        in0=v_sb[:, k, col:col + 64], scalar1=za[:, k:k + 1])

        def pass2(p, c):
            # out^T chunk = sum_s (V/Z)^T-slices @ P^T
            pvp = ps_ab.tile([P, CH], f32, name="pvp", tag="ab")
            # heads interleaved: consecutive MMs target disjoint col groups
            # (tile_position 0 / 64) so the PE runs both heads' chains
            # CONCURRENTLY (col-tiling). PSUM started-state is tracked per
            # partition x zero-region, and the chains are partition-disjoint,
            # so interleaved start/stop flags are safe.
            us = [u for u in range(16) if _win(u)[0] < (c + 1) * CH]
            for n, u in enumerate(us):
                tl, w, _ = _win(u)
                lo = max(tl, c * CH)
                wid = (c + 1) * CH - lo
                for h in range(2):
                    nc.tensor.matmul(
                        pvp[h * 64:(h + 1) * 64, lo - c * CH:],
                        v_sb[:, u, (2 * p + h) * 64:(2 * p + h + 1) * 64],
                        ptt[(p, h)][:, pt_off[u] + lo - tl:
                                    pt_off[u] + lo - tl + wid],
                        start=(n == 0), stop=(n == len(us) - 1),
                        tile_position=(0, h * 64))
            nc.vector.tensor_copy(att_sb[:, p, c * CH:(c + 1) * CH], pvp)

        for p in range(NPAIR):
            if p >= 1:
                zg_fetch(p - 1)
            qk_pass1(p)
            if p == 0:
                v_loop()
            late_weight_dmas(p)
            if p >= 1:
                zfix(p - 1)
                pass2(p - 1, 0)
                pass2(p - 1, 1)
        zg_fetch(NPAIR - 1)
        zfix(NPAIR - 1)

        # ---- tail: last pair's pass-2 chunk-wise, proj + residual 1 + LN2
        # per 512-token chunk, then FFN per chunk.
        for c in range(2):
            pass2(NPAIR - 1, c)
            for i in range(c * 4, c * 4 + 4):
                py = ps_mm.tile([P, C], f32, name="py", tag="pq")
                for p in range(NPAIR):
                    nc.tensor.matmul(py, att_sb[:, p, i * P:(i + 1) * P],
                                     wp_sb[:, p, :], start=(p == 0), stop=(p == 2))
                # residual uses the bf16 x copy (x's bf16 quantization adds
                # ~0.3% rel err vs the 2e-2 gate; saves 12KB SBUF + a DMA)
                nc.vector.tensor_add(out=r_sb[:, i, :], in0=py,
                                     in1=x_sb[:, i, :])
                if nz["bproj"]:
                    nc.vector.tensor_add(out=r_sb[:, i, :], in0=r_sb[:, i, :],
                                         in1=bproj_sb)
                st2 = lnp.tile([P, 6], f32, name="st2")
                nc.vector.bn_stats(out=st2, in_=r_sb[:, i, :])
                mv2 = lnp.tile([P, 2], f32, name="mv2")
                nc.vector.bn_aggr(out=mv2, in_=st2)
                rs2 = lnp.tile([P, 1], f32, name="rs2")
                nc.scalar.activation(out=rs2, in_=mv2[:, 1:2], func=AF.Sqrt,
                                     bias=eps_sb)
                nc.vector.reciprocal(out=rs2, in_=rs2)
                h2b = lnp.tile([P, C], bf16, name="h2b")
                nc.vector.tensor_scalar(out=h2b, in0=r_sb[:, i, :],
                                        scalar1=mv2[:, 0:1], scalar2=rs2,
                                        op0=ALU.subtract, op1=ALU.mult)
                for cc in range(3):
                    tp2 = ps_ab.tile([P, P], bf16, name="tp2", tag="ab")
                    nc.tensor.transpose(tp2, h2b[:, cc * P:(cc + 1) * P],
                                        ident_sb)
                    nc.vector.tensor_copy(h2T[:, cc, i * P:(i + 1) * P], tp2)

        # ---- FFN + residual 2 + store. Two 512-wide hidden chunks share a
        # [P,1024] PSUM tile (ps_s is idle by now) and, when b1 is zero, a
        # single relu — halving the FFN1 matmul->relu round-trips.
        for c in range(2):
            hid = []
            for cb2 in range(6):
                ph = ps_s.tile([P, 1024], f32, name="ph", tag="sp")
                for half in range(2):
                    cb = 2 * cb2 + half
                    for cc in range(3):
                        nc.tensor.matmul(
                            ph[:, half * CH:(half + 1) * CH],
                            w1_sb[:, cc, cb * P:(cb + 1) * P],
                            h2T[:, cc, c * CH:(c + 1) * CH],
                            start=(cc == 0), stop=(cc == 2))
                ht_ = hidp.tile([P, 2, CH], bf16, name=f"ht{c}_{cb2}",
                                tag="hid")
                if nz["b1"]:
                    for half in range(2):
                        cb = 2 * cb2 + half
                        nc.scalar.activation(
                            out=ht_[:, half, :],
                            in_=ph[:, half * CH:(half + 1) * CH],
                            func=AF.Relu, bias=b1_sb[:, cb:cb + 1])
                else:
                    nc.scalar.activation(out=ht_, in_=ph, func=AF.Relu)
                hid.append(ht_)
            for jj in range(4):
                i = c * 4 + jj
                pf = ps_mm.tile([P, C], f32, name="pf", tag="pq")
                for cb in range(12):
                    nc.tensor.matmul(pf, hid[cb // 2][:, cb % 2,
                                                      jj * P:(jj + 1) * P],
                                     w2_sb[:, cb, :], start=(cb == 0),
                                     stop=(cb == 11))
                ot = outp.tile([P, C], f32, name="ot")
                nc.vector.tensor_add(out=ot, in0=pf, in1=r_sb[:, i, :])
                if nz["b2"]:
                    nc.vector.tensor_add(out=ot, in0=ot, in1=b2_sb)
                nc.sync.dma_start(out=out_d[i * P:(i + 1) * P, :], in_=ot)

    nc.compile()
    return nc


def _prep_inputs(inputs):
    """Host-side: fold gains into weights, build per-core input maps."""
    x = np.asarray(inputs["x"], np.float32)
    g1 = np.asarray(inputs["g1"], np.float32)
    be1 = np.asarray(inputs["be1"], np.float32)
    g2 = np.asarray(inputs["g2"], np.float32)
    be2 = np.asarray(inputs["be2"], np.float32)
    # attention scale folded into wq so masks added to S psum stay at NEG
    wq = np.asarray(inputs["wq"], np.float32) * g1[None, :, None] * SCALE
    wk = np.asarray(inputs["wk"], np.float32) * g1[None, :, None]
    wv = np.asarray(inputs["wv"], np.float32) * g1[None, :, None]
    bq = np.einsum("c,hcd->hd", be1,
                   np.asarray(inputs["wq"], np.float32)) * SCALE
    bk = np.einsum("c,hcd->hd", be1, np.asarray(inputs["wk"], np.float32))
    bv = np.einsum("c,hcd->hd", be1, np.asarray(inputs["wv"], np.float32))
    wp = np.asarray(inputs["w_proj"], np.float32)
    bproj = np.asarray(inputs["b_proj"], np.float32)
    w1 = np.asarray(inputs["w1"], np.float32) * g2[:, None]
    b1 = np.asarray(inputs["b1"], np.float32) + be2 @ np.asarray(
        inputs["w1"], np.float32)
    w2 = np.asarray(inputs["w2"], np.float32)
    b2 = np.asarray(inputs["b2"], np.float32)

    nz = dict(bqk=bool(np.any(bq) or np.any(bk)), bv=bool(np.any(bv)),
              bproj=bool(np.any(bproj)), b2=bool(np.any(b2)),
              b1=bool(np.any(b1)))

    # wqk [128, cc, pair, qk, col]: lhsT chunks (c-partition, head-pair cols)
    wqk = np.zeros((P, 3, NPAIR, 2, P), BF16)
    for pr in range(NPAIR):
        for qk, w in ((0, wq), (1, wk)):
            pair = np.concatenate([w[2 * pr], w[2 * pr + 1]], axis=1)  # [C,128]
            wqk[:, :, pr, qk, :] = pair.reshape(3, P, P).transpose(1, 0, 2)
    wv_all = np.concatenate([wv[h] for h in range(H)], axis=1)  # [C, 384]
    wv_pre = wv_all.reshape(3, P, C).transpose(1, 0, 2).astype(BF16)
    wp_pre = wp.reshape(3, P, C).transpose(1, 0, 2).astype(BF16)
    w1_pre = w1.reshape(3, P, D4).transpose(1, 0, 2).astype(BF16)
    w2_pre = w2.reshape(12, P, C).transpose(1, 0, 2).astype(BF16)
    b1_pre = np.ascontiguousarray(b1.reshape(12, P).T).astype(np.float32)

    ident = np.eye(P, dtype=BF16)
    negi = (np.eye(P) * NEG).astype(BF16)
    sl = np.tril(np.ones((P, P)), -1).astype(BF16)  # strict lower: s > t

    common = dict(wqk=wqk, wv=wv_pre, wp=wp_pre, w1=w1_pre, w2=w2_pre,
                  b1=b1_pre, ident=ident, negi=negi, trilm=sl)
    if nz["bqk"]:
        bqk = np.zeros((P, NPAIR, 2), np.float32)
        for pr in range(NPAIR):
            bqk[:, pr, 0] = np.concatenate([bq[2 * pr], bq[2 * pr + 1]])
            bqk[:, pr, 1] = np.concatenate([bk[2 * pr], bk[2 * pr + 1]])
        common["bqk"] = bqk
    if nz["bv"]:
        common["bv"] = np.broadcast_to(
            np.concatenate([bv[h] for h in range(H)]), (P, C)).copy()
    if nz["bproj"]:
        common["bproj"] = np.broadcast_to(bproj, (P, C)).copy()
    if nz["b2"]:
        common["b2"] = np.broadcast_to(b2, (P, C)).copy()

    ones = np.ones((P, P), BF16)
    zeros = np.zeros((P, P), BF16)
    in_maps = []
    for core in range(N_CORES):
        b, par = core // 2, core % 2
        perm = list(range(par, NT, 2)) + list(range(1 - par, NT, 2))
        xt = x[b].reshape(NT, P, C)[perm]  # [NT, P, C]
        m = dict(common)
        m["x_wide"] = np.ascontiguousarray(
            xt.transpose(1, 0, 2)).astype(BF16)
        # q3 boundary (s other, t owned): phys 2u+(1-par) vs 2u+par:
        #   par=0: s odd > t even at boundary -> invalid -> mask ON
        m["q3m"] = ones if par == 0 else zeros
        m["swapsel"] = np.full((P, 16), par, np.uint8)
        in_maps.append(m)
    return in_maps, nz


def _purge_neff_cache():
    # libneuronxla's NEFF cache is keyed on the HLO module hash, which does
    # not cover the BIR carried in backend_config -- a stale kernel body can
    # be silently reused across program edits. Purge before compiling.
    import glob, os, shutil
    for d in glob.glob(os.path.expanduser(
            "~/.neuron-compile-cache/*/MODULE_*")):
        try:
            shutil.rmtree(d, ignore_errors=True)
        except OSError:
            pass


def kernel(**inputs):
    import os
    pt_fp8 = os.environ.get("KPT8", "0") == "1"
    _purge_neff_cache()
    in_maps, nz = _prep_inputs(inputs)
    key = (pt_fp8, tuple(sorted(nz.items())))
    if key not in _PROG_CACHE:
        _PROG_CACHE[key] = _build_program(nz, pt_fp8)
    nc = _PROG_CACHE[key]

    from concourse.bass_utils import run_bass_kernel_spmd
    res = run_bass_kernel_spmd(nc, in_maps, core_ids=list(range(N_CORES)))

    out = np.empty((B, T, C), np.float32)
    for core in range(N_CORES):
        b, par = core // 2, core % 2
        tiles = out[b].reshape(NT, P, C)
        tiles[par::2] = res.results[core]["out"].reshape(8, P, C)
    return out



# revision 27
# speedup vs baseline: 1.2619x; 1.1971x over previous
# Trainium2 Bass kernel for nn_Block_9483287789889 (dense transformer block).
#
# Sharding (8 cores): 2 cores per batch (B=4). Host permutes each batch's
# 2048 tokens into [owned 8x128-tiles (interleaved) | other 8 tiles] so both
# cores of a pair run an IDENTICAL program (SPMD) with all per-core variation
# carried by input data (token permutation + boundary-mask patterns).
# Attention (softmax over the QUERY axis -> per-key normalizers Z[s]) is
# computed in S^T layout [s_partition, t_free], s-tile-major: one wide
# [128, 1024] PSUM strip per (pair, head, s_tile) and a single exp whose
# accum_out yields that s_tile's Z partial directly. Each core computes exp
# only over its owned-query half; the pair's Z partials are combined with a
# per-pair AllGather (cheaper than AllReduce in latency) + local add.
# Program order: all pairs' pass-1 + collectives are issued before any
# pass-2 so the collectives overlap pass-1 of later pairs.
import sys

if "/opt/trn_rl_repo" not in sys.path:
    sys.path.insert(0, "/opt/trn_rl_repo")

import numpy as np
import ml_dtypes

BF16 = ml_dtypes.bfloat16

B, T, C, H, HS = 4, 2048, 384, 6, 64
D4 = 4 * C  # 1536
EPS = 1e-5
NPAIR = H // 2  # 3 head-pairs
P = 128
NT = T // P  # 16 token tiles
CH = 512
OWN = T // 2  # 1024 owned tokens per core
NEG = -30.0
SCALE = float(C) ** -0.5
N_CORES = 8
GROUPS = [[0, 1], [2, 3], [4, 5], [6, 7]]

_PROG_CACHE = {}


def _win(u):
    """Owned-t window of local s_tile u: (t_lo, width, mask_kind).

    Local layout: t-tiles 0..7 owned (phys 2u+par), s-tiles 0..7 owned,
    8..15 other (phys 2k+(1-par)). Owned s_tile u: valid t >= u*128 with a
    true tril diag at t-tile u. Other s_tile 8+k: valid t >= k*128 with an
    all-or-nothing boundary block at t-tile k (q3m: ones iff par==0).
    """
    if u < 8:
        return u * P, OWN - u * P, "tril"
    k = u - 8
    return k * P, OWN - k * P, "q3m"


def _pt_layout():
    """Column offset of each s_tile's strip inside ptt, per (pair, head)."""
    off = {}
    pos = 0
    for u in range(16):
        _, w, _ = _win(u)
        off[u] = pos
        pos += w
    return off, pos  # pos = 9216


def _build_program(nz, pt_fp8=False):
    """nz: dict of nonzero-bias flags (bqk, bv, bproj, b2)."""
    import concourse.bass as bass
    import concourse.bacc as bacc
    import concourse.mybir as mybir
    from concourse.tile import TileContext
    from contextlib import ExitStack

    f32 = mybir.dt.float32
    bf16 = mybir.dt.bfloat16
    pt_dt = mybir.dt.float8e4 if pt_fp8 else bf16
    AF = mybir.ActivationFunctionType
    ALU = mybir.AluOpType

    nc = bacc.Bacc("TRN2", target_bir_lowering=False)

    # x shipped partition-major ([P, tile, C]) so each quarter is ONE fat
    # DMA (3KB/partition contiguous) instead of 16 trickling 768B-descriptor
    # tile DMAs (which starved LN1 until ~40us on the baseline trace).
    xw_d = nc.dram_tensor("x_wide", [P, NT, C], bf16, kind="ExternalInput")
    wqk_d = nc.dram_tensor("wqk", [P, 3, NPAIR, 2, P], bf16, kind="ExternalInput")
    wv_d = nc.dram_tensor("wv", [P, 3, C], bf16, kind="ExternalInput")
    wp_d = nc.dram_tensor("wp", [P, 3, C], bf16, kind="ExternalInput")
    w1_d = nc.dram_tensor("w1", [P, 3, D4], bf16, kind="ExternalInput")
    w2_d = nc.dram_tensor("w2", [P, 12, C], bf16, kind="ExternalInput")
    b1_d = nc.dram_tensor("b1", [P, 12], f32, kind="ExternalInput")
    ident_d = nc.dram_tensor("ident", [P, P], bf16, kind="ExternalInput")
    negi_d = nc.dram_tensor("negi", [P, P], bf16, kind="ExternalInput")
    tril_d = nc.dram_tensor("trilm", [P, P], bf16, kind="ExternalInput")
    q3m_d = nc.dram_tensor("q3m", [P, P], bf16, kind="ExternalInput")
    swapsel_d = nc.dram_tensor("swapsel", [P, 16], mybir.dt.uint8,
                               kind="ExternalInput")
    if nz["bqk"]:
        bqk_d = nc.dram_tensor("bqk", [P, NPAIR, 2], f32, kind="ExternalInput")
    if nz["bv"]:
        bv_d = nc.dram_tensor("bv", [P, C], f32, kind="ExternalInput")
    if nz["bproj"]:
        bproj_d = nc.dram_tensor("bproj", [P, C], f32, kind="ExternalInput")
    if nz["b2"]:
        b2_d = nc.dram_tensor("b2", [P, C], f32, kind="ExternalInput")
    zin_d = nc.dram_tensor("zin", [NPAIR, 2, P, 16], f32)
    zout_d = nc.dram_tensor("zout", [NPAIR, 2, 2, P, 16], f32)
    # bf16 output, partition-major, stored in two fat DMAs (f32 per-tile
    # stores left an ~11us serial drain at the end of the kernel)
    out_d = nc.dram_tensor("out", [P, 8, C], bf16, kind="ExternalOutput")

    pt_off, pt_cols = _pt_layout()

    with TileContext(nc) as tc, ExitStack() as ctx:
        cst = ctx.enter_context(tc.tile_pool(name="const", bufs=1))
        persist = ctx.enter_context(tc.tile_pool(name="persist", bufs=1))
        lnp = ctx.enter_context(tc.tile_pool(name="ln", bufs=4))
        qkp = ctx.enter_context(tc.tile_pool(name="qk", bufs=2))
        ptp = ctx.enter_context(
            tc.tile_pool(name="ptp", bufs=(6 if pt_fp8 else 4)))
        zp = ctx.enter_context(tc.tile_pool(name="zp", bufs=3))
        hidp = ctx.enter_context(tc.tile_pool(name="hid", bufs=7))
        outp = ctx.enter_context(tc.tile_pool(name="outp", bufs=2))
        ps_s = ctx.enter_context(tc.tile_pool(name="ps_s", bufs=2, space="PSUM"))
        ps_mm = ctx.enter_context(tc.tile_pool(name="ps_mm", bufs=2, space="PSUM"))
        ps_ab = ctx.enter_context(tc.tile_pool(name="ps_ab", bufs=2, space="PSUM"))

        # ---- constants / weights. ident + the fat x DMAs go on the SP
        # HWDGE queue (compute can start ~2us in); everything else via the
        # otherwise-idle gpsimd SWDGE queue, ordered by first use.
        ident_sb = cst.tile([P, P], bf16)
        nc.sync.dma_start(out=ident_sb, in_=ident_d[:])
        x_sb = persist.tile([P, NT, C], bf16)
        for qtr in range(4):
            nc.sync.dma_start(out=x_sb[:, qtr * 4:(qtr + 1) * 4, :],
                              in_=xw_d[:, qtr * 4:(qtr + 1) * 4, :])
        wv_sb = cst.tile([P, 3, C], bf16)
        nc.gpsimd.dma_start(out=wv_sb, in_=wv_d[:])
        wqk_sb = cst.tile([P, 3, NPAIR, 2, P], bf16)
        nc.gpsimd.dma_start(out=wqk_sb, in_=wqk_d[:])
        negi_sb = cst.tile([P, P], bf16)
        nc.gpsimd.dma_start(out=negi_sb, in_=negi_d[:])
        mask_sb = {}
        for nm, d in (("tril", tril_d), ("q3m", q3m_d)):
            m = cst.tile([P, P], bf16, name=f"m_{nm}")
            nc.gpsimd.dma_start(out=m, in_=d[:])
            mask_sb[nm] = m
        swapsel_sb = cst.tile([P, 16], mybir.dt.uint8)
        nc.gpsimd.dma_start(out=swapsel_sb, in_=swapsel_d[:])
        if nz["bqk"]:
            bqk_sb = cst.tile([P, NPAIR, 2], f32)
            nc.gpsimd.dma_start(out=bqk_sb, in_=bqk_d[:])
        if nz["bv"]:
            bv_sb = cst.tile([P, C], f32)
            nc.gpsimd.dma_start(out=bv_sb, in_=bv_d[:])
        # tail-stage weights: tiles declared here, DMAs issued later (inside
        # the attention pipeline) so they don't crowd x/wv/wqk at startup
        wp_sb = cst.tile([P, 3, C], bf16)
        b1_sb = cst.tile([P, 12], f32)
        w1_sb = cst.tile([P, 3, D4], bf16)
        w2_sb = cst.tile([P, 12, C], bf16)
        if nz["bproj"]:
            bproj_sb = cst.tile([P, C], f32)
        if nz["b2"]:
            b2_sb = cst.tile([P, C], f32)

        def late_weight_dmas(stage):
            if stage == 0:
                nc.gpsimd.dma_start(out=wp_sb, in_=wp_d[:])
                nc.gpsimd.dma_start(out=b1_sb, in_=b1_d[:])
                if nz["bproj"]:
                    nc.gpsimd.dma_start(out=bproj_sb, in_=bproj_d[:])
                if nz["b2"]:
                    nc.gpsimd.dma_start(out=b2_sb, in_=b2_d[:])
            else:
                nc.gpsimd.dma_start(out=w1_sb, in_=w1_d[:])
                nc.gpsimd.dma_start(out=w2_sb, in_=w2_d[:])

        eps_sb = cst.tile([P, 1], f32)
        nc.vector.memset(eps_sb, EPS)

        # ---- PE warm-up: the HAM clock gate holds the PE at 1.2 GHz until
        # it sees ~3.4us of sustained matmul activity. The first ~11us of
        # the kernel (preamble + x DMA) have no real PE work, so without
        # this the whole LN1/QK/pass-1 ramp runs at half clock. Burn ~4us
        # of dummy ident matmuls (regular MATMULs -- transpose-mode does
        # not count as PE-busy for HAM) into a dead-end PSUM tile.
        warm_ps = ps_mm.tile([P, P], f32, name="warm", tag="pq")
        for _ in range(24):
            nc.tensor.matmul(warm_ps, ident_sb, ident_sb, start=True,
                             stop=True)

        hT = persist.tile([P, 3, T], bf16)       # normalized x, transposed
        v_sb = persist.tile([P, NT, C], bf16)    # V (later scaled to V/Z)
        att_sb = persist.tile([P, NPAIR, OWN], bf16)  # attention out^T
        h2T = persist.tile([P, 3, OWN], bf16)    # LN2 out, transposed
        r_sb = persist.tile([P, 8, C], f32)      # residual-1 tiles (owned)

        # ---- LN1 + transpose into hT (x arrives in 4 fat quarter-DMAs).
        # Sqrt is batched over 8 tiles' variances at a time: every Sqrt is
        # then dependency-ordered BEFORE the first pass-1 Exp, so the ACT
        # table (Sqrt and Exp live in different sets) loads exactly once
        # per phase instead of toggling 1.3us reloads mid-exp-stream.
        # LN1 reads a host-provided bf16 copy of x (half the DMA bytes, 2x
        # DVE modes); the residual path uses the f32 owned-half copy.
        xts = [x_sb[:, i, :] for i in range(NT)]
        mvs = persist.tile([P, NT, 2], f32)
        rs16 = persist.tile([P, NT], f32)
        groups = [(0, 2), (2, 4), (4, 8), (8, 12), (12, 16)]
        for lo, hi in groups:
            for i in range(lo, hi):
                st = lnp.tile([P, 6], f32, name="st")
                nc.vector.bn_stats(out=st, in_=xts[i])
                nc.vector.bn_aggr(out=mvs[:, i, :], in_=st)
            nc.scalar.activation(out=rs16[:, lo:hi],
                                 in_=mvs[:, lo:hi, 1],
                                 func=AF.Sqrt, bias=eps_sb)
            nc.vector.reciprocal(out=rs16[:, lo:hi], in_=rs16[:, lo:hi])
            for i in range(lo, hi):
                hb = lnp.tile([P, C], bf16, name="hb")
                nc.vector.tensor_scalar(out=hb, in0=xts[i],
                                        scalar1=mvs[:, i, 0:1],
                                        scalar2=rs16[:, i:i + 1],
                                        op0=ALU.subtract, op1=ALU.mult)
                for cc in range(3):
                    tp = ps_ab.tile([P, P], bf16, name="tp", tag="ab")
                    nc.tensor.transpose(tp, hb[:, cc * P:(cc + 1) * P],
                                        ident_sb)
                    # let the scheduler balance these across ACT/DVE
                    nc.any.tensor_copy(hT[:, cc, i * P:(i + 1) * P], tp)

        def v_loop():
            # V for all heads (lhsT = hT chunk, rhs = wv). Emitted after
            # QK(0)+pass-1(0) so it fills PE/DVE idle time while ACT runs
            # pair 0's exps (V isn't read until pass-2 of pair 0).
            for i in range(NT):
                pv = ps_mm.tile([P, C], f32, name="pv", tag="pq")
                for cc in range(3):
                    nc.tensor.matmul(pv, hT[:, cc, i * P:(i + 1) * P],
                                     wv_sb[:, cc, :], start=(cc == 0),
                                     stop=(cc == 2))
                if nz["bv"]:
                    nc.vector.tensor_add(out=v_sb[:, i, :], in0=pv, in1=bv_sb)
                else:
                    nc.vector.tensor_copy(v_sb[:, i, :], pv)

        # ---- attention, software-pipelined per head-pair:
        #   QK(p) -> pass-1(p) -> Z AllGather(p) issued, then while it (and
        #   the next pair's pass-1) runs: Z-combine(p-1) + pass-2(p-1).
        # ptt slot rotation (bufs=4) matches this program order.
        qt = {}
        kt = {}
        ptt = {}

        def qk_pass1(p):
            qt[p] = qkp.tile([P, OWN], bf16, name=f"qt{p}", tag="qt")
            kt[p] = qkp.tile([P, T], bf16, name=f"kt{p}", tag="kt")
            # qt first, then kt chunks in order: the s_tile-0 exp only needs
            # qt + kt chunk 0, so the first exp can start ASAP
            for qk, dst, nch in ((0, qt[p], 2), (1, kt[p], 4)):
                for c in range(nch):
                    pq = ps_mm.tile([P, CH], f32, name="pq")
                    for cc in range(3):
                        nc.tensor.matmul(pq, wqk_sb[:, cc, p, qk, :],
                                         hT[:, cc, c * CH:(c + 1) * CH],
                                         start=(cc == 0), stop=(cc == 2))
                    if nz["bqk"]:
                        nc.vector.tensor_scalar(
                            out=dst[:, c * CH:(c + 1) * CH], in0=pq,
                            scalar1=bqk_sb[:, p, qk:qk + 1], scalar2=None,
                            op0=ALU.add)
                    else:
                        nc.vector.tensor_copy(dst[:, c * CH:(c + 1) * CH], pq)

            # both heads' score strips computed CONCURRENTLY as row-tiled
            # matmul pairs: head h occupies array rows h*64..h*64+63 (the
            # head-dim contraction is only 64 deep), halving scores PE time.
            # The full-row mask matmuls briefly serialize the two streams.
            zls = {}
            for h in range(2):
                ptt[(p, h)] = ptp.tile([P, pt_cols], pt_dt,
                                       name=f"pt{p}_{h}", tag="pt")
                zls[h] = zp.tile([P, 16], f32, name=f"zl{p}_{h}", tag="zl",
                                 bufs=6)
            for u in range(16):
                tl, w, mk = _win(u)
                sps = [ps_s.tile([P, 1024], f32, name=f"sp{h}", tag="sp")
                       for h in range(2)]
                nmm = (w + CH - 1) // CH
                for j in range(nmm):
                    wj = min(CH, w - j * CH)
                    for h in range(2):
                        hb_ = h * 64
                        nc.tensor.matmul(
                            sps[h][:, j * CH:j * CH + wj],
                            kt[p][hb_:hb_ + 64, u * P:(u + 1) * P],
                            qt[p][hb_:hb_ + 64, tl + j * CH:tl + j * CH + wj],
                            start=True, stop=(j > 0),
                            tile_position=(hb_, 0))
                    if j == 0:
                        # -30 on the masked part of the boundary/diag
                        # tile (first 128 cols), accumulated via PE.
                        for h in range(2):
                            nc.tensor.matmul(sps[h][:, 0:P], negi_sb,
                                             mask_sb[mk], start=False,
                                             stop=True)
                for h in range(2):
                    nc.scalar.activation(
                        out=ptt[(p, h)][:, pt_off[u]:pt_off[u] + w],
                        in_=sps[h][:, :w], func=AF.Exp,
                        accum_out=zls[h][:, u:u + 1])
            for h in range(2):
                nc.sync.dma_start(out=zin_d[p, h], in_=zls[h])
            nc.gpsimd.collective_compute(
                "AllGather", ALU.bypass, replica_groups=GROUPS,
                ins=[zin_d[p]], outs=[zout_d[p]])

        zgs = {}

        def zg_fetch(p):
            # issued early so these DMAs sit ahead of the NEXT pair's zin
            # on the in-order Pool queue (they only wait on collective p)
            zg = zp.tile([P, 2, 2, 16], f32, name=f"zg{p}", tag="zg")
            nc.gpsimd.dma_start(
                out=zg, in_=zout_d[p].rearrange("r h p z -> p r h z"))
            zgs[p] = zg

        def zfix(p):
            # combine Z partials from the AllGather, scale V cols by 1/Z
            zg = zgs[p]
            for h in range(2):
                # Z_local = mine + swap8(partner). With g0/g1 in replica
                # order and A = g0 + swap8(g1): par==0 -> A, par==1 ->
                # swap8(A); selected via the swapsel input (==par).
                za = zp.tile([P, 16], f32, name=f"za{p}_{h}", tag="za")
                nc.vector.tensor_tensor(out=za[:, 0:8], in0=zg[:, 0, h, 0:8],
                                        in1=zg[:, 1, h, 8:16], op=ALU.add)
                nc.vector.tensor_tensor(out=za[:, 8:16], in0=zg[:, 0, h, 8:16],
                                        in1=zg[:, 1, h, 0:8], op=ALU.add)
                zb = zp.tile([P, 16], f32, name=f"zb{p}_{h}", tag="zb")
                nc.vector.tensor_copy(zb[:, 0:8], za[:, 8:16])
                nc.vector.tensor_copy(zb[:, 8:16], za[:, 0:8])
                nc.vector.copy_predicated(za, swapsel_sb, zb)
                nc.vector.reciprocal(out=za, in_=za)
                col = (2 * p + h) * 64
                for k in range(16):
                    nc.vector.tensor_scalar_mul(
                        out=v_sb[:, k, col:col + 64],
                        in0=v_sb[:, k, col:col + 64], scalar1=za[:, k:k + 1])

        def pass2(p, c):
            # out^T chunk = sum_s (V/Z)^T-slices @ P^T
            pvp = ps_ab.tile([P, CH], f32, name="pvp", tag="ab")
            # heads interleaved: consecutive MMs target disjoint col groups
            # (tile_position 0 / 64) so the PE runs both heads' chains
            # CONCURRENTLY (col-tiling). PSUM started-state is tracked per
            # partition x zero-region, and the chains are partition-disjoint,
            # so interleaved start/stop flags are safe.
            us = [u for u in range(16) if _win(u)[0] < (c + 1) * CH]
            for n, u in enumerate(us):
                tl, w, _ = _win(u)
                lo = max(tl, c * CH)
                wid = (c + 1) * CH - lo
                for h in range(2):
                    nc.tensor.matmul(
                        pvp[h * 64:(h + 1) * 64, lo - c * CH:],
                        v_sb[:, u, (2 * p + h) * 64:(2 * p + h + 1) * 64],
                        ptt[(p, h)][:, pt_off[u] + lo - tl:
                                    pt_off[u] + lo - tl + wid],
                        start=(n == 0), stop=(n == len(us) - 1),
                        tile_position=(0, h * 64))
            nc.vector.tensor_copy(att_sb[:, p, c * CH:(c + 1) * CH], pvp)

        for p in range(NPAIR):
            if p >= 1:
                zg_fetch(p - 1)
            qk_pass1(p)
            if p == 0:
                v_loop()
            late_weight_dmas(p)
            if p >= 1:
                zfix(p - 1)
                pass2(p - 1, 0)
                pass2(p - 1, 1)
        zg_fetch(NPAIR - 1)
        zfix(NPAIR - 1)

        # ---- tail: last pair's pass-2 chunk-wise, proj + residual 1 + LN2
        # per 512-token chunk, then FFN per chunk.
        for c in range(2):
            pass2(NPAIR - 1, c)
            for i in range(c * 4, c * 4 + 4):
                py = ps_mm.tile([P, C], f32, name="py", tag="pq")
                for p in range(NPAIR):
                    nc.tensor.matmul(py, att_sb[:, p, i * P:(i + 1) * P],
                                     wp_sb[:, p, :], start=(p == 0), stop=(p == 2))
                # residual uses the bf16 x copy (x's bf16 quantization adds
                # ~0.3% rel err vs the 2e-2 gate; saves 12KB SBUF + a DMA)
                nc.vector.tensor_add(out=r_sb[:, i, :], in0=py,
                                     in1=x_sb[:, i, :])
                if nz["bproj"]:
                    nc.vector.tensor_add(out=r_sb[:, i, :], in0=r_sb[:, i, :],
                                         in1=bproj_sb)
                st2 = lnp.tile([P, 6], f32, name="st2")
                nc.vector.bn_stats(out=st2, in_=r_sb[:, i, :])
                mv2 = lnp.tile([P, 2], f32, name="mv2")
                nc.vector.bn_aggr(out=mv2, in_=st2)
                rs2 = lnp.tile([P, 1], f32, name="rs2")
                nc.scalar.activation(out=rs2, in_=mv2[:, 1:2], func=AF.Sqrt,
                                     bias=eps_sb)
                nc.vector.reciprocal(out=rs2, in_=rs2)
                h2b = lnp.tile([P, C], bf16, name="h2b")
                nc.vector.tensor_scalar(out=h2b, in0=r_sb[:, i, :],
                                        scalar1=mv2[:, 0:1], scalar2=rs2,
                                        op0=ALU.subtract, op1=ALU.mult)
                for cc in range(3):
                    tp2 = ps_ab.tile([P, P], bf16, name="tp2", tag="ab")
                    nc.tensor.transpose(tp2, h2b[:, cc * P:(cc + 1) * P],
                                        ident_sb)
                    nc.vector.tensor_copy(h2T[:, cc, i * P:(i + 1) * P], tp2)

        # ---- FFN + residual 2 + store. Two 512-wide hidden chunks share a
        # [P,1024] PSUM tile (ps_s is idle by now) and, when b1 is zero, a
        # single relu — halving the FFN1 matmul->relu round-trips.
        for c in range(2):
            hid = []
            for cb2 in range(6):
                ph = ps_s.tile([P, 1024], f32, name="ph", tag="sp")
                for half in range(2):
                    cb = 2 * cb2 + half
                    for cc in range(3):
                        nc.tensor.matmul(
                            ph[:, half * CH:(half + 1) * CH],
                            w1_sb[:, cc, cb * P:(cb + 1) * P],
                            h2T[:, cc, c * CH:(c + 1) * CH],
                            start=(cc == 0), stop=(cc == 2))
                ht_ = hidp.tile([P, 2, CH], bf16, name=f"ht{c}_{cb2}",
                                tag="hid")
                if nz["b1"]:
                    for half in range(2):
                        cb = 2 * cb2 + half
                        nc.scalar.activation(
                            out=ht_[:, half, :],
                            in_=ph[:, half * CH:(half + 1) * CH],
                            func=AF.Relu, bias=b1_sb[:, cb:cb + 1])
                else:
                    nc.scalar.activation(out=ht_, in_=ph, func=AF.Relu)
                hid.append(ht_)
            ot = outp.tile([P, 4, C], bf16, name="ot")
            for jj in range(4):
                i = c * 4 + jj
                pf = ps_mm.tile([P, C], f32, name="pf", tag="pq")
                for cb in range(12):
                    nc.tensor.matmul(pf, hid[cb // 2][:, cb % 2,
                                                      jj * P:(jj + 1) * P],
                                     w2_sb[:, cb, :], start=(cb == 0),
                                     stop=(cb == 11))
                nc.vector.tensor_add(out=ot[:, jj, :], in0=pf,
                                     in1=r_sb[:, i, :])
                if nz["b2"]:
                    nc.vector.tensor_add(out=ot[:, jj, :], in0=ot[:, jj, :],
                                         in1=b2_sb)
            nc.sync.dma_start(out=out_d[:, c * 4:c * 4 + 4, :], in_=ot)

    nc.compile()
    return nc


def _prep_inputs(inputs):
    """Host-side: fold gains into weights, build per-core input maps."""
    x = np.asarray(inputs["x"], np.float32)
    g1 = np.asarray(inputs["g1"], np.float32)
    be1 = np.asarray(inputs["be1"], np.float32)
    g2 = np.asarray(inputs["g2"], np.float32)
    be2 = np.asarray(inputs["be2"], np.float32)
    # attention scale folded into wq so masks added to S psum stay at NEG
    wq = np.asarray(inputs["wq"], np.float32) * g1[None, :, None] * SCALE
    wk = np.asarray(inputs["wk"], np.float32) * g1[None, :, None]
    wv = np.asarray(inputs["wv"], np.float32) * g1[None, :, None]
    bq = np.einsum("c,hcd->hd", be1,
                   np.asarray(inputs["wq"], np.float32)) * SCALE
    bk = np.einsum("c,hcd->hd", be1, np.asarray(inputs["wk"], np.float32))
    bv = np.einsum("c,hcd->hd", be1, np.asarray(inputs["wv"], np.float32))
    wp = np.asarray(inputs["w_proj"], np.float32)
    bproj = np.asarray(inputs["b_proj"], np.float32)
    w1 = np.asarray(inputs["w1"], np.float32) * g2[:, None]
    b1 = np.asarray(inputs["b1"], np.float32) + be2 @ np.asarray(
        inputs["w1"], np.float32)
    w2 = np.asarray(inputs["w2"], np.float32)
    b2 = np.asarray(inputs["b2"], np.float32)

    nz = dict(bqk=bool(np.any(bq) or np.any(bk)), bv=bool(np.any(bv)),
              bproj=bool(np.any(bproj)), b2=bool(np.any(b2)),
              b1=bool(np.any(b1)))

    # wqk [128, cc, pair, qk, col]: lhsT chunks (c-partition, head-pair cols)
    wqk = np.zeros((P, 3, NPAIR, 2, P), BF16)
    for pr in range(NPAIR):
        for qk, w in ((0, wq), (1, wk)):
            pair = np.concatenate([w[2 * pr], w[2 * pr + 1]], axis=1)  # [C,128]
            wqk[:, :, pr, qk, :] = pair.reshape(3, P, P).transpose(1, 0, 2)
    wv_all = np.concatenate([wv[h] for h in range(H)], axis=1)  # [C, 384]
    wv_pre = wv_all.reshape(3, P, C).transpose(1, 0, 2).astype(BF16)
    wp_pre = wp.reshape(3, P, C).transpose(1, 0, 2).astype(BF16)
    w1_pre = w1.reshape(3, P, D4).transpose(1, 0, 2).astype(BF16)
    w2_pre = w2.reshape(12, P, C).transpose(1, 0, 2).astype(BF16)
    b1_pre = np.ascontiguousarray(b1.reshape(12, P).T).astype(np.float32)

    ident = np.eye(P, dtype=BF16)
    negi = (np.eye(P) * NEG).astype(BF16)
    sl = np.tril(np.ones((P, P)), -1).astype(BF16)  # strict lower: s > t

    common = dict(wqk=wqk, wv=wv_pre, wp=wp_pre, w1=w1_pre, w2=w2_pre,
                  b1=b1_pre, ident=ident, negi=negi, trilm=sl)
    if nz["bqk"]:
        bqk = np.zeros((P, NPAIR, 2), np.float32)
        for pr in range(NPAIR):
            bqk[:, pr, 0] = np.concatenate([bq[2 * pr], bq[2 * pr + 1]])
            bqk[:, pr, 1] = np.concatenate([bk[2 * pr], bk[2 * pr + 1]])
        common["bqk"] = bqk
    if nz["bv"]:
        common["bv"] = np.broadcast_to(
            np.concatenate([bv[h] for h in range(H)]), (P, C)).copy()
    if nz["bproj"]:
        common["bproj"] = np.broadcast_to(bproj, (P, C)).copy()
    if nz["b2"]:
        common["b2"] = np.broadcast_to(b2, (P, C)).copy()

    ones = np.ones((P, P), BF16)
    zeros = np.zeros((P, P), BF16)
    in_maps = []
    for core in range(N_CORES):
        b, par = core // 2, core % 2
        perm = list(range(par, NT, 2)) + list(range(1 - par, NT, 2))
        xt = x[b].reshape(NT, P, C)[perm]  # [NT, P, C]
        m = dict(common)
        m["x_wide"] = np.ascontiguousarray(
            xt.transpose(1, 0, 2)).astype(BF16)
        # q3 boundary (s other, t owned): phys 2u+(1-par) vs 2u+par:
        #   par=0: s odd > t even at boundary -> invalid -> mask ON
        m["q3m"] = ones if par == 0 else zeros
        m["swapsel"] = np.full((P, 16), par, np.uint8)
        in_maps.append(m)
    return in_maps, nz


def _purge_neff_cache():
    # libneuronxla's NEFF cache is keyed on the HLO module hash, which does
    # not cover the BIR carried in backend_config -- a stale kernel body can
    # be silently reused across program edits. Purge before compiling.
    import glob, os, shutil
    for d in glob.glob(os.path.expanduser(
            "~/.neuron-compile-cache/*/MODULE_*")):
        try:
            shutil.rmtree(d, ignore_errors=True)
        except OSError:
            pass


def kernel(**inputs):
    import os
    pt_fp8 = os.environ.get("KPT8", "0") == "1"
    _purge_neff_cache()
    in_maps, nz = _prep_inputs(inputs)
    key = (pt_fp8, tuple(sorted(nz.items())))
    if key not in _PROG_CACHE:
        _PROG_CACHE[key] = _build_program(nz, pt_fp8)
    nc = _PROG_CACHE[key]

    from concourse.bass_utils import run_bass_kernel_spmd
    res = run_bass_kernel_spmd(nc, in_maps, core_ids=list(range(N_CORES)))

    out = np.empty((B, T, C), np.float32)
    for core in range(N_CORES):
        b, par = core // 2, core % 2
        tiles = out[b].reshape(NT, P, C)
        core_out = np.asarray(res.results[core]["out"])  # [P, 8, C] bf16
        tiles[par::2] = core_out.transpose(1, 0, 2).astype(np.float32)
    return out

